# revision 8
# baseline (speedup 1.0000x reference)
"""Trainium2 Bass kernel for nn_CausalSelfAttention_17368847745133.

Sharding (8 NeuronCores): core (b, g) = batch b in 0..3 x head-group g in
0..1 (8 heads each; Megatron column/row-parallel c_attn / c_proj).  The host
passes x[b].T so every device matmul runs transpose-free:

  qT/kT [512,2048] : matmul(lhsT=W_q|k slice, rhs=xT)      (transposed proj)
  V     [2048,512] : matmul(lhsT=xT tile, rhs=W_v slice)   (natural layout)
  S^T   [k,q]      : matmul(lhsT=kT head, rhs=qT head)     (d=64 contraction,
                     head pairs packed on PE row-groups 0-63 / 64-127,
                     concurrent row-tiled matmuls)
  P^T   = exp((S^T + causal_mask) / 8)
  U'    [65,q]     : matmul(lhsT=[V_head|ones], rhs=P^T)   row 64 = denom
  y^T   = U'[0:64] * bcast(qm / denom) + t3A  (t3A = host-built pad blend)
  oT    [1024,2048]: matmul(lhsT=W_proj rows, rhs=y^T); written fp16; host
                     sums the two group partials, transposes, adds b_proj.

Rows q >= l[b] reproduce the reference exactly: the reference's additive
-1e8 mask makes softmax on those rows exactly uniform, so y = mean_k v =
(mean_t x) @ W_v -- computed on the HOST and shipped as the t3A blend table
(t3A[:,hp,q] = (1-qm[q]) * ypad).  All matmuls run in bf16; softmax
statistics and normalization stay fp32.

Scheduling: the q/k projection chunks for head-pair hp+1 are interleaved
into hp's attention j-loop (the exp on the ACT engine paces attention, so
the PE has bubbles to fill), and the output projection for q-block j runs
right after head-pair 3 normalizes that block.
"""

from collections import deque

import ml_dtypes
import numpy as np

import concourse.bass as bass
import concourse.mybir as mybir
import concourse.tile as tile
from concourse import bacc
from concourse.bass_utils import run_bass_kernel_spmd

P = 128
B, T, C = 4, 2048, 1024
H, D = 16, 64
G = 2
HPG = H // G     # 8 heads per core
CG = HPG * D     # 512 channels per group
F32 = mybir.dt.float32
F16 = mybir.dt.float16
BF16 = mybir.dt.bfloat16
SCALE = 0.125    # 1/sqrt(64)

_CACHED_NC = None


def build_nc():
    nc = bacc.Bacc(trn_type="TRN2", target_bir_lowering=False)

    xT = nc.dram_tensor("xT", [C, T], BF16, kind="ExternalInput")
    wq = nc.dram_tensor("wq", [P, 8, CG], BF16, kind="ExternalInput")
    wk = nc.dram_tensor("wk", [P, 8, CG], BF16, kind="ExternalInput")
    wv = nc.dram_tensor("wv", [P, 8, CG], BF16, kind="ExternalInput")
    wp = nc.dram_tensor("wp", [P, 4, C], BF16, kind="ExternalInput")
    qmA = nc.dram_tensor("qmA", [2, 16, 512], F32, kind="ExternalInput")
    m01 = nc.dram_tensor("m01", [P, P], BF16, kind="ExternalInput")
    t3A = nc.dram_tensor("t3A", [P, 4, T], BF16, kind="ExternalInput")
    oT = nc.dram_tensor("oT", [C, T], F16, kind="ExternalOutput")

    with tile.TileContext(nc) as tc:
        with tc.tile_pool(name="big", bufs=1) as big, \
             tc.tile_pool(name="qk", bufs=1) as qkpool, \
             tc.tile_pool(name="vp", bufs=1) as vpool, \
             tc.tile_pool(name="w", bufs=4) as wpool, \
             tc.tile_pool(name="pt", bufs=4) as ptpool, \
             tc.tile_pool(name="misc", bufs=1) as misc, \
             tc.tile_pool(name="norm", bufs=3) as norm, \
             tc.tile_pool(name="ob", bufs=3) as obpool, \
             tc.tile_pool(name="rdram", bufs=2, space="DRAM") as rdram, \
             tc.tile_pool(name="psS", bufs=2, space="PSUM") as psS, \
             tc.tile_pool(name="psU", bufs=4, space="PSUM") as psU:

            # ---- constants / small inputs ----
            m01_sb = misc.tile([P, P], BF16, tag="m01")
            qmA_sb = misc.tile([2, 16, 512], F32, tag="qmA")
            t3A_sb = misc.tile([P, 4, T], BF16, tag="t3A")
            dend = rdram.tile([32, 512], F32, tag="dend")

            # ---- input DMAs, in consumption order ----
            w_tiles = {}
            for nm, wd in [("wv", wv), ("wq", wq), ("wk", wk)]:
                wt = wpool.tile([P, 8, CG], BF16, tag="w", name=nm)
                w_tiles[nm] = wt
            wp_v = wpool.tile([P, 4, C], BF16, tag="w", name="wpv")

            nc.sync.dma_start(w_tiles["wv"], wv[:])
            xT_bf = big.tile([P, 8, T], BF16, tag="big")
            # token-block DMAs so the V projection can start early
            for tb in range(8):
                ts = slice(tb * 256, (tb + 1) * 256)
                for kt in range(8):
                    nc.sync.dma_start(
                        xT_bf[:, kt, ts], xT[kt * P:(kt + 1) * P, ts])
            nc.sync.dma_start(w_tiles["wq"], wq[:])
            nc.sync.dma_start(w_tiles["wk"], wk[:])
            nc.sync.dma_start(wp_v, wp[:])
            nc.sync.dma_start(m01_sb, m01[:])
            nc.sync.dma_start(qmA_sb, qmA[:])
            nc.sync.dma_start(t3A_sb, t3A[:])

            # ---- Phase B: V projection (token-block pipelined) ----
            V_sb = vpool.tile([P, 16, HPG, D + 1], BF16, tag="V")
            nc.vector.memset(V_sb[:, :, :, D:D + 1], 1.0)
            wv_sb = w_tiles["wv"]
            for tb in range(8):
                ps = psS.tile([P, 2, 512], F32, tag="psS", name=f"v{tb}")
                for half in range(2):
                    tt = 2 * tb + half
                    for kt in range(8):
                        nc.tensor.matmul(
                            ps[:, half],
                            xT_bf[:, kt, tt * P:(tt + 1) * P],
                            wv_sb[:, kt, :],
                            start=(kt == 0), stop=(kt == 7))
                for half in range(2):
                    tt = 2 * tb + half
                    nc.scalar.copy(
                        V_sb[:, tt, :, 0:D],
                        ps[:, half].rearrange("p (h d) -> p h d", h=HPG))

            qT_sb = qkpool.tile([P, 4, T], BF16, tag="qT")
            kT_sb = qkpool.tile([P, 4, T], BF16, tag="kT")
            yT_sb = big.tile([P, 4, T], BF16, tag="yT")

            def qk_chunk(hp, side, nbh):
                """Project q (side=0) or k (side=1) for head-pair hp,
                token half nbh (1024 tokens, as 2x512 sub-chunks)."""
                w_sb = w_tiles["wq" if side == 0 else "wk"]
                dst = qT_sb if side == 0 else kT_sb
                for nb2 in range(2):
                    ps = psU.tile([P, 512], F32, tag="psU",
                                  name=f"qk{hp}_{side}_{nbh}_{nb2}")
                    cs = slice(nbh * 1024 + nb2 * 512,
                               nbh * 1024 + (nb2 + 1) * 512)
                    for kt in range(8):
                        nc.tensor.matmul(
                            ps,
                            w_sb[:, kt, hp * P:(hp + 1) * P],
                            xT_bf[:, kt, cs],
                            start=(kt == 0), stop=(kt == 7))
                    nc.vector.tensor_copy(dst[:, hp, cs], ps)

            # global qk chunk order: (hp, side k first, then q), nbh pairs
            chunk_q = deque()
            for hp in range(4):
                for nbh in range(2):
                    chunk_q.append((hp, 1, nbh))   # k side
                    chunk_q.append((hp, 0, nbh))   # q side
            # issue the first two chunks (k0/q0 of hp0) up front
            for _ in range(2):
                qk_chunk(*chunk_q.popleft())

            def out_proj(qb):
                for mt in range(8):
                    ps = psU.tile([P, 512], F32, tag="psU",
                                  name=f"op{qb}_{mt}")
                    for ct in range(4):
                        nc.tensor.matmul(
                            ps,
                            wp_v[:, ct, mt * P:(mt + 1) * P],
                            yT_sb[:, ct, qb * 512:(qb + 1) * 512],
                            start=(ct == 0), stop=(ct == 3))
                    ot = obpool.tile([P, 512], F16, tag="ob")
                    nc.scalar.copy(ot, ps)
                    nc.sync.dma_start(
                        oT[mt * P:(mt + 1) * P, qb * 512:(qb + 1) * 512],
                        ot)

            # ---- Phase C: attention ----
            for hp in range(4):
                for j in range(4):
                    nkt = 4 * (j + 1)
                    Upr = [psU.tile([P, 512], F32, tag="psU",
                                    name=f"U_{hp}_{j}_{par}")
                           for par in range(2)]

                    def s_exp(kt, j=j, hp=hp):
                        dlt = 128 * kt - 512 * j
                        c0 = max(dlt, 0)
                        ss = psS.tile([P, 2, 512], F32, tag="psS")
                        for par in range(2):
                            p0 = par * D
                            nc.tensor.matmul(
                                ss[:, par, c0:512],
                                kT_sb[p0:p0 + D, hp, kt * P:(kt + 1) * P],
                                qT_sb[p0:p0 + D, hp,
                                      512 * j + c0:512 * (j + 1)],
                                start=True, stop=True)
                        pt = ptpool.tile([P, 2, 512], BF16, tag="pt")
                        nc.scalar.activation(
                            pt[:, :, c0:512], ss[:, :, c0:512],
                            mybir.ActivationFunctionType.Exp,
                            bias=0.0, scale=SCALE)
                        if dlt >= 0:
                            nc.vector.tensor_mul(
                                out=pt[:, :, c0:c0 + P],
                                in0=pt[:, :, c0:c0 + P],
                                in1=m01_sb[:, None, :].to_broadcast(
                                    [P, 2, P]))
                        return pt, c0

                    def pv(kt, pt, c0, hp=hp):
                        for par in range(2):
                            h = 2 * hp + par
                            nc.tensor.matmul(
                                Upr[par][0:D + 1, c0:512],
                                V_sb[:, kt, h, :],
                                pt[:, par, c0:512],
                                start=(kt == 0), stop=(kt == nkt - 1))

                    prev = None
                    for kt in range(nkt):
                        cur = s_exp(kt)
                        if prev is not None:
                            pv(kt - 1, *prev)
                        prev = cur
                    pv(nkt - 1, *prev)

                    # denominators + unnormalized stash
                    r0 = hp * 8 + 2 * j
                    blk = slice(512 * j, 512 * (j + 1))
                    den2 = norm.tile([P, 512], F32, tag="dtf", name="den2")
                    for par in range(2):
                        U = Upr[par]
                        dtf = norm.tile([P, 512], F32, tag="dtf")
                        nc.vector.tensor_copy(dtf[D:D + 1, :], U[D:D + 1, :])
                        nc.sync.dma_start(den2[par:par + 1, :],
                                          dtf[D:D + 1, :])
                    nc.vector.tensor_copy(yT_sb[0:D, hp, blk], Upr[0][0:D, :])
                    ytmp = norm.tile([D, 512], BF16, tag="ytmp")
                    nc.vector.tensor_copy(ytmp, Upr[1][0:D, :])
                    nc.sync.dma_start(yT_sb[D:P, hp, blk], ytmp)

                    # rb = qm/den broadcast to 128 partitions (via DRAM hop)
                    dq2 = norm.tile([P, 512], F32, tag="dtf", name="dq2")
                    nc.vector.reciprocal_approx_fast(dq2[0:2, :],
                                                     den2[0:2, :])
                    nc.vector.tensor_mul(out=dq2[0:2, :], in0=dq2[0:2, :],
                                         in1=qmA_sb[:, hp * 4 + j, :])
                    nc.sync.dma_start(dend[r0:r0 + 2, :], dq2[0:2, :])
                    rb = norm.tile([P, 512], F32, tag="rb")
                    for par in range(2):
                        row = dend[r0 + par:r0 + par + 1, :]
                        src = bass.AP(
                            tensor=row.tensor, offset=row.offset,
                            ap=[[0, D]] + list(row.ap[1:]))
                        nc.sync.dma_start(rb[par * D:(par + 1) * D, :], src)
                    ys = yT_sb[:, hp, blk]
                    nc.vector.tensor_mul(out=ys, in0=ys, in1=rb)
                    nc.vector.tensor_add(out=ys, in0=ys,
                                         in1=t3A_sb[:, hp, blk])

                    if chunk_q:
                        qk_chunk(*chunk_q.popleft())
                    if hp == 3:
                        out_proj(j)

    nc.compile()
    return nc


def _bf(a):
    return np.ascontiguousarray(np.asarray(a)).astype(ml_dtypes.bfloat16)


def _prep_inputs(x, l, W_attn, b_attn, W_proj, b_proj):
    x = np.asarray(x, dtype=np.float32)
    W_attn = np.asarray(W_attn, dtype=np.float32)
    W_proj = np.asarray(W_proj, dtype=np.float32)
    lv = np.asarray(l).astype(np.int64)

    m01 = np.where(np.arange(P)[:, None] > np.arange(P)[None, :],
                   0.0, 1.0).astype(ml_dtypes.bfloat16)

    in_maps = []
    for b in range(B):
        xTb = np.ascontiguousarray(x[b].T).astype(ml_dtypes.bfloat16)
        xbar = x[b].mean(axis=0)                       # [C]
        lb = int(np.clip(lv[b], 0, T))
        qrow = (np.arange(T) < lb).astype(np.float32)
        qmA = np.empty((2, 16, 512), dtype=np.float32)
        for hp in range(4):
            for j in range(4):
                for par in range(2):
                    qmA[par, hp * 4 + j] = qrow[512 * j:512 * (j + 1)]
        for g in range(2):
            cs = slice(g * CG, (g + 1) * CG)
            wqg = _bf(
                W_attn[:, 0:C][:, cs].reshape(8, P, CG).transpose(1, 0, 2))
            wkg = _bf(
                W_attn[:, C:2 * C][:, cs].reshape(8, P, CG).transpose(1, 0, 2))
            wvg = _bf(
                W_attn[:, 2 * C:3 * C][:, cs].reshape(8, P, CG).transpose(1, 0, 2))
            wpg = _bf(
                W_proj[cs, :].reshape(4, P, C).transpose(1, 0, 2))
            # pad blend table: t3A[64*par+d, hp, q] = (1-qrow[q])*ypad[2hp+par][d]
            ypad = (xbar @ W_attn[:, 2 * C:3 * C][:, cs]).reshape(HPG, D)
            t3A = np.zeros((P, 4, T), dtype=np.float32)
            nq = 1.0 - qrow
            for hp in range(4):
                for par in range(2):
                    t3A[par * D:(par + 1) * D, hp, :] = (
                        ypad[2 * hp + par][:, None] * nq[None, :])
            in_maps.append({
                "xT": xTb, "wq": wqg, "wk": wkg, "wv": wvg, "wp": wpg,
                "qmA": qmA, "m01": m01, "t3A": _bf(t3A),
            })
    return in_maps


def kernel(x, l, W_attn, b_attn, W_proj, b_proj, _want_profile=False):
    global _CACHED_NC
    if _CACHED_NC is None:
        _CACHED_NC = build_nc()
    nc = _CACHED_NC

    b_attn = np.asarray(b_attn, dtype=np.float32)
    b_proj = np.asarray(b_proj, dtype=np.float32)
    assert not np.any(b_attn), "nonzero b_attn not supported by this kernel"

    in_maps = _prep_inputs(x, l, W_attn, b_attn, W_proj, b_proj)
    res = run_bass_kernel_spmd(nc, in_maps, core_ids=list(range(8)),
                               trace=_want_profile)

    out = np.empty((B, T, C), dtype=np.float32)
    for b in range(B):
        acc = (res.results[2 * b]["oT"].astype(np.float32)
               + res.results[2 * b + 1]["oT"].astype(np.float32))
        out[b] = acc.T + b_proj[None, :]
    if _want_profile:
        return out, res
    return out


# revision 17
# speedup vs baseline: 1.0833x; 1.0833x over previous
"""Trainium2 Bass kernel for nn_CausalSelfAttention_17368847745133.

Sharding (8 NeuronCores): core (b, g) = batch b in 0..3 x head-group g in
0..1 (8 heads each; Megatron column/row-parallel c_attn / c_proj).  The host
passes x[b].T (token-block-major) so every device matmul is transpose-free:

  qT/kT [512,2048] : matmul(lhsT=W_q|k slice, rhs=xT)      (transposed proj)
  V     [2048,512] : matmul(lhsT=xT tile, rhs=W_v slice)   (natural layout)
  S^T   [k,q]      : matmul(lhsT=kT head, rhs=qT head)     (d=64 contraction,
                     head pairs packed on PE row-groups 0-63 / 64-127,
                     concurrent row-tiled matmuls)
  P^T   = exp((S^T + causal_mask) / 8)
  U'    [65,q]     : matmul(lhsT=[V_head|ones], rhs=P^T)   row 64 = denom
  y^T   = U'[0:64] * bcast(qm / denom) + t3A  (t3A = host-built pad blend)
  oT    [1024,2048]: matmul(lhsT=W_proj rows, rhs=y^T); written fp16; host
                     sums the two group partials, transposes, adds b_proj.

Rows q >= l[b] reproduce the reference exactly: softmax there is uniform,
y = mean_k v = (mean_t x) @ W_v -- computed on the HOST and shipped via the
t3A blend table.  The denominator rows ride along the U copies (row 64),
land in yT/ytmp as bf16, and get gathered by DMA for a fast approximate
reciprocal.  All matmuls run bf16; normalization is fp32.

Scheduling: q/k projection chunks for head-pair hp+1 are interleaved into
hp's attention j-loop (ACT-engine exp paces attention, so the PE has
bubbles), and the output projection for q-block j runs right after
head-pair 3 normalizes that block.
"""

from collections import deque

import ml_dtypes
import numpy as np

import concourse.bass as bass
import concourse.mybir as mybir
import concourse.tile as tile
from concourse import bacc
from concourse.bass_utils import run_bass_kernel_spmd

P = 128
B, T, C = 4, 2048, 1024
H, D = 16, 64
G = 2
HPG = H // G     # 8 heads per core
CG = HPG * D     # 512 channels per group
F32 = mybir.dt.float32
F16 = mybir.dt.float16
BF16 = mybir.dt.bfloat16
SCALE = 0.125    # 1/sqrt(64)

_CACHED_NC = None


def build_nc(debug=False):
    nc = bacc.Bacc(trn_type="TRN2", target_bir_lowering=False)

    # xTB[tb, p, kt, t]: token-block-major xT so one DMA lands a full
    # 256-token block (contiguous 4KB per partition row)
    xTB = nc.dram_tensor("xTB", [8, P, 8, 256], BF16, kind="ExternalInput")
    wq = nc.dram_tensor("wq", [P, 8, CG], BF16, kind="ExternalInput")
    wk = nc.dram_tensor("wk", [P, 8, CG], BF16, kind="ExternalInput")
    wv = nc.dram_tensor("wv", [P, 8, CG], BF16, kind="ExternalInput")
    wp = nc.dram_tensor("wp", [P, 4, C], BF16, kind="ExternalInput")
    qmA = nc.dram_tensor("qmA", [2, 16, 512], BF16, kind="ExternalInput")
    m01 = nc.dram_tensor("m01", [P, P], BF16, kind="ExternalInput")
    t3A = nc.dram_tensor("t3A", [P, 4, T], BF16, kind="ExternalInput")
    oT = nc.dram_tensor("oT", [C, T], F16, kind="ExternalOutput")
    if debug:
        d_den = nc.dram_tensor("d_den", [32, 512], F32, kind="ExternalOutput")
        d_rb = nc.dram_tensor("d_rb", [32, 512], F32, kind="ExternalOutput")
        d_yT = nc.dram_tensor("d_yT", [P, 4, T], BF16, kind="ExternalOutput")

    with tile.TileContext(nc) as tc:
        with tc.tile_pool(name="big", bufs=1) as big, \
             tc.tile_pool(name="qk", bufs=1) as qkpool, \
             tc.tile_pool(name="vp", bufs=1) as vpool, \
             tc.tile_pool(name="w", bufs=4) as wpool, \
             tc.tile_pool(name="pt", bufs=4) as ptpool, \
             tc.tile_pool(name="misc", bufs=1) as misc, \
             tc.tile_pool(name="norm", bufs=3) as norm, \
             tc.tile_pool(name="ob", bufs=3) as obpool, \
             tc.tile_pool(name="rdram", bufs=2, space="DRAM") as rdram, \
             tc.tile_pool(name="psS", bufs=3, space="PSUM") as psS, \
             tc.tile_pool(name="psU", bufs=2, space="PSUM") as psU:

            # ---- constants / small inputs ----
            m01_sb = misc.tile([P, P], BF16, tag="m01")
            qmA_sb = misc.tile([2, 16, 512], BF16, tag="qmA")
            t3A_sb = misc.tile([P, 4, T], BF16, tag="t3A")
            dend = rdram.tile([32, 512], F32, tag="dend")

            # ---- input DMAs, in consumption order ----
            w_tiles = {}
            for nm, wd in [("wv", wv), ("wq", wq), ("wk", wk)]:
                wt = wpool.tile([P, 8, CG], BF16, tag="w", name=nm)
                w_tiles[nm] = wt
            wp_v = wpool.tile([P, 4, C], BF16, tag="w", name="wpv")

            nc.sync.dma_start(w_tiles["wv"], wv[:])
            xT_bf = big.tile([P, 8, T], BF16, tag="big")
            for tb in range(8):
                nc.sync.dma_start(
                    xT_bf[:, :, tb * 256:(tb + 1) * 256], xTB[tb])
            nc.sync.dma_start(w_tiles["wq"], wq[:])
            nc.sync.dma_start(w_tiles["wk"], wk[:])
            nc.sync.dma_start(wp_v, wp[:])
            nc.sync.dma_start(m01_sb, m01[:])
            nc.sync.dma_start(qmA_sb, qmA[:])
            nc.sync.dma_start(t3A_sb, t3A[:])

            # ---- Phase B: V projection (token-block pipelined) ----
            V_sb = vpool.tile([P, 16, HPG, D + 1], BF16, tag="V")
            nc.vector.memset(V_sb[:, :, :, D:D + 1], 1.0)
            wv_sb = w_tiles["wv"]
            for tb in range(8):
                ps = psS.tile([P, 2, 512], F32, tag="psS", name=f"v{tb}")
                for half in range(2):
                    tt = 2 * tb + half
                    for kt in range(8):
                        nc.tensor.matmul(
                            ps[:, half],
                            xT_bf[:, kt, tt * P:(tt + 1) * P],
                            wv_sb[:, kt, :],
                            start=(kt == 0), stop=(kt == 7))
                for half in range(2):
                    tt = 2 * tb + half
                    nc.scalar.copy(
                        V_sb[:, tt, :, 0:D],
                        ps[:, half].rearrange("p (h d) -> p h d", h=HPG))

            qT_sb = qkpool.tile([P, 4, T], BF16, tag="qT")
            kT_sb = qkpool.tile([P, 4, T], BF16, tag="kT")
            yT_sb = big.tile([P, 4, T], BF16, tag="yT")

            def qk_chunk(hp, side, nbh):
                """Project q (side=0) or k (side=1) for head-pair hp,
                token half nbh (1024 tokens)."""
                w_sb = w_tiles["wq" if side == 0 else "wk"]
                dst = qT_sb if side == 0 else kT_sb
                ps = psS.tile([P, 2, 512], F32, tag="psS",
                              name=f"qk{hp}_{side}_{nbh}")
                for nb2 in range(2):
                    cs = slice(nbh * 1024 + nb2 * 512,
                               nbh * 1024 + (nb2 + 1) * 512)
                    for kt in range(8):
                        nc.tensor.matmul(
                            ps[:, nb2],
                            w_sb[:, kt, hp * P:(hp + 1) * P],
                            xT_bf[:, kt, cs],
                            start=(kt == 0), stop=(kt == 7))
                cs2 = slice(nbh * 1024, (nbh + 1) * 1024)
                nc.vector.tensor_copy(
                    dst[:, hp, cs2], ps.rearrange("p a b -> p (a b)"))

            # global qk chunk order: per hp k then q, nbh-interleaved
            chunk_q = deque()
            for hp in range(4):
                for nbh in range(2):
                    chunk_q.append((hp, 1, nbh))   # k side
                    chunk_q.append((hp, 0, nbh))   # q side
            for _ in range(2):
                qk_chunk(*chunk_q.popleft())

            def out_proj(qb):
                for mt in range(8):
                    ps = psU.tile([P, 512], F32, tag="psU",
                                  name=f"op{qb}_{mt}")
                    for ct in range(4):
                        nc.tensor.matmul(
                            ps,
                            wp_v[:, ct, mt * P:(mt + 1) * P],
                            yT_sb[:, ct, qb * 512:(qb + 1) * 512],
                            start=(ct == 0), stop=(ct == 3))
                    ot = obpool.tile([P, 512], F16, tag="ob")
                    if qb < 3:
                        nc.vector.tensor_copy(ot, ps)
                    else:
                        nc.scalar.copy(ot, ps)
                    nc.gpsimd.dma_start(
                        oT[mt * P:(mt + 1) * P, qb * 512:(qb + 1) * 512],
                        ot)

            # ---- Phase C: attention ----
            for hp in range(4):
                for j in range(4):
                    nkt = 4 * (j + 1)
                    Upr = [psU.tile([P, 512], F32, tag="psU",
                                    name=f"U_{hp}_{j}_{par}")
                           for par in range(2)]

                    def s_exp(kt, j=j, hp=hp):
                        dlt = 128 * kt - 512 * j
                        c0 = max(dlt, 0)
                        ss = psS.tile([P, 2, 512], F32, tag="psS")
                        for par in range(2):
                            p0 = par * D
                            nc.tensor.matmul(
                                ss[:, par, c0:512],
                                kT_sb[p0:p0 + D, hp, kt * P:(kt + 1) * P],
                                qT_sb[p0:p0 + D, hp,
                                      512 * j + c0:512 * (j + 1)],
                                start=True, stop=True)
                        pt = ptpool.tile([P, 2, 512], BF16, tag="pt")
                        nc.scalar.activation(
                            pt[:, :, c0:512], ss[:, :, c0:512],
                            mybir.ActivationFunctionType.Exp,
                            bias=0.0, scale=SCALE)
                        if dlt >= 0:
                            nc.vector.tensor_mul(
                                out=pt[:, :, c0:c0 + P],
                                in0=pt[:, :, c0:c0 + P],
                                in1=m01_sb[:, None, :].to_broadcast(
                                    [P, 2, P]))
                        return pt, c0

                    def pv(kt, pt, c0, hp=hp):
                        for par in range(2):
                            h = 2 * hp + par
                            nc.tensor.matmul(
                                Upr[par][0:D + 1, c0:512],
                                V_sb[:, kt, h, :],
                                pt[:, par, c0:512],
                                start=(kt == 0), stop=(kt == nkt - 1))

                    prev = None
                    for kt in range(nkt):
                        cur = s_exp(kt)
                        if prev is not None:
                            pv(kt - 1, *prev)
                        prev = cur
                    pv(nkt - 1, *prev)

                    # stash unnormalized y; denominators ride row 64
                    r0 = hp * 8 + 2 * j
                    blk = slice(512 * j, 512 * (j + 1))
                    nc.vector.tensor_copy(yT_sb[0:D + 1, hp, blk],
                                          Upr[0][0:D + 1, :])
                    ytmp = norm.tile([D + 1, 512], BF16, tag="ytmp")
                    nc.vector.tensor_copy(ytmp, Upr[1][0:D + 1, :])
                    # gather the two bf16 denom rows (before ytmp's DMA
                    # overwrites yT row 64), then shift par1 rows into place
                    den2b = norm.tile([2, 512], BF16, tag="den2b")
                    nc.gpsimd.dma_start(den2b[0:1, :],
                                        yT_sb[D:D + 1, hp, blk])
                    nc.gpsimd.dma_start(den2b[1:2, :], ytmp[D:D + 1, :])
                    nc.gpsimd.dma_start(yT_sb[D:P, hp, blk], ytmp[0:D, :])

                    # rb = qm/den broadcast to 128 partitions (via DRAM hop)
                    dq2 = norm.tile([2, 512], F32, tag="dq2")
                    dqo = norm.tile([2, 512], F32, tag="dqo")
                    nc.vector.tensor_copy(dq2, den2b)
                    nc.vector.reciprocal_approx_fast(dqo, dq2)
                    nc.vector.tensor_mul(out=dqo, in0=dqo,
                                         in1=qmA_sb[:, hp * 4 + j, :])
                    nc.gpsimd.dma_start(dend[r0:r0 + 2, :], dqo)
                    if debug:
                        nc.gpsimd.dma_start(d_den[r0:r0 + 2, :], dq2)
                        nc.gpsimd.dma_start(d_rb[r0:r0 + 2, :], dqo)
                    rb = norm.tile([P, 512], F32, tag="rb")
                    for par in range(2):
                        row = dend[r0 + par:r0 + par + 1, :]
                        src = bass.AP(
                            tensor=row.tensor, offset=row.offset,
                            ap=[[0, D]] + list(row.ap[1:]))
                        nc.gpsimd.dma_start(rb[par * D:(par + 1) * D, :], src)
                    ys = yT_sb[:, hp, blk]
                    nc.vector.tensor_mul(out=ys, in0=ys, in1=rb)
                    nc.vector.tensor_add(out=ys, in0=ys,
                                         in1=t3A_sb[:, hp, blk])

                    if chunk_q:
                        qk_chunk(*chunk_q.popleft())
                    if hp == 3:
                        out_proj(j)
            if debug:
                nc.gpsimd.dma_start(d_yT[:], yT_sb)

    nc.compile()
    return nc


def _bf(a):
    return np.ascontiguousarray(np.asarray(a)).astype(ml_dtypes.bfloat16)


def _prep_inputs(x, l, W_attn, b_attn, W_proj, b_proj):
    x = np.asarray(x, dtype=np.float32)
    W_attn = np.asarray(W_attn, dtype=np.float32)
    W_proj = np.asarray(W_proj, dtype=np.float32)
    lv = np.asarray(l).astype(np.int64)

    m01 = np.where(np.arange(P)[:, None] > np.arange(P)[None, :],
                   0.0, 1.0).astype(ml_dtypes.bfloat16)

    in_maps = []
    for b in range(B):
        xTb = np.ascontiguousarray(x[b].T).astype(ml_dtypes.bfloat16)
        # [kt, p, tb, t] -> [tb, p, kt, t]
        xTB = np.ascontiguousarray(
            xTb.reshape(8, P, 8, 256).transpose(2, 1, 0, 3))
        xbar = x[b].mean(axis=0)                       # [C]
        lb = int(np.clip(lv[b], 0, T))
        qrow = (np.arange(T) < lb).astype(np.float32)
        qmA = np.empty((2, 16, 512), dtype=np.float32)  # cast below
        for hp in range(4):
            for j in range(4):
                for par in range(2):
                    qmA[par, hp * 4 + j] = qrow[512 * j:512 * (j + 1)]
        for g in range(2):
            cs = slice(g * CG, (g + 1) * CG)
            wqg = _bf(
                W_attn[:, 0:C][:, cs].reshape(8, P, CG).transpose(1, 0, 2))
            wkg = _bf(
                W_attn[:, C:2 * C][:, cs].reshape(8, P, CG).transpose(1, 0, 2))
            wvg = _bf(
                W_attn[:, 2 * C:3 * C][:, cs].reshape(8, P, CG).transpose(1, 0, 2))
            wpg = _bf(
                W_proj[cs, :].reshape(4, P, C).transpose(1, 0, 2))
            # pad blend: t3A[64*par+d, hp, q] = (1-qrow[q])*ypad[2hp+par][d]
            ypad = (xbar @ W_attn[:, 2 * C:3 * C][:, cs]).reshape(HPG, D)
            t3A = np.zeros((P, 4, T), dtype=np.float32)
            nq = 1.0 - qrow
            for hp in range(4):
                for par in range(2):
                    t3A[par * D:(par + 1) * D, hp, :] = (
                        ypad[2 * hp + par][:, None] * nq[None, :])
            in_maps.append({
                "xTB": xTB, "wq": wqg, "wk": wkg, "wv": wvg, "wp": wpg,
                "qmA": _bf(qmA), "m01": m01, "t3A": _bf(t3A),
            })
    return in_maps


def kernel(x, l, W_attn, b_attn, W_proj, b_proj, _want_profile=False):
    global _CACHED_NC
    if _CACHED_NC is None:
        _CACHED_NC = build_nc()
    nc = _CACHED_NC

    b_attn = np.asarray(b_attn, dtype=np.float32)
    b_proj = np.asarray(b_proj, dtype=np.float32)
    assert not np.any(b_attn), "nonzero b_attn not supported by this kernel"

    in_maps = _prep_inputs(x, l, W_attn, b_attn, W_proj, b_proj)
    res = run_bass_kernel_spmd(nc, in_maps, core_ids=list(range(8)),
                               trace=_want_profile)

    out = np.empty((B, T, C), dtype=np.float32)
    for b in range(B):
        acc = (res.results[2 * b]["oT"].astype(np.float32)
               + res.results[2 * b + 1]["oT"].astype(np.float32))
        out[b] = acc.T + b_proj[None, :]
    if _want_profile:
        return out, res
    return out
